# revision 51
# baseline (speedup 1.0000x reference)
"""Causal MHA (batch=4, seq=2048, dim=1024, 16 heads x 64) on 8 TRN2 NeuronCores.

Sharding: core c handles batch b = c//2 and head-group g = c%2 (8 heads).
Each core computes QKV projections for its heads, causal attention, and a
partial output projection over its 512 features. The host sums the two
partial projections per batch and transposes back.

All matmuls run in bf16 (fp32 PSUM accumulate); softmax runs without max
subtraction (logits are bounded ~|8|), with the row sums produced by an
extra ones-column appended to V during the PV matmul. Diagonal tiles only
stream their causally-valid columns through the PE; the softmax reciprocal
runs on DVE and is broadcast to 128 partitions by a single rank-2 matmul
per head pair.
"""
import sys

sys.path.insert(0, "/opt/trn_rl_repo")

import json
import numpy as np
import ml_dtypes
from contextlib import ExitStack

import concourse.bass as bass
import concourse.tile as tile
from concourse import mybir
from concourse import bass_utils as _bu
from concourse.bass_utils import run_bass_kernel_spmd

LDW_OPT = False  # walrus ldw-opt rejects bass-emitted Ldweights outright

BF16 = mybir.dt.bfloat16
F32 = mybir.dt.float32
Exp = mybir.ActivationFunctionType.Exp

DIM = 1024
SEQ = 2048
NH = 16          # total heads
HPC = 8          # heads per core
DH = 64          # head dim
SCALE = DH ** -0.5
NCORES = 8
FPC = HPC * DH   # features per core = 512
NKT = SEQ // 128   # 16 k-tiles of 128
NQC = SEQ // 512   # 4 q-chunks of 512
VSTRIDE = DH + 2   # 66: V columns per head incl. ones col + pad

_WALRUS_PATCHED = False


def _patch_walrus_wait_limit():
    """This container's walrus rejects >1 sem wait per instruction
    (CoreV3 setupSyncWait). Tile's tail drain carries one wait per live
    proc; split the extras into preceding single-wait Drain carriers at
    BIR-JSON serialization time."""
    global _WALRUS_PATCHED
    if _WALRUS_PATCHED:
        return
    _WALRUS_PATCHED = True

    if LDW_OPT:
        orig_run = _bu.run_command

        def run_patched(cmd, *a, **k):
            cmd = ["--enable-ldw-opt=true" if c == "--enable-ldw-opt=false" else c
                   for c in cmd]
            return orig_run(cmd, *a, **k)

        _bu.run_command = run_patched

    orig = bass.Bass.to_json_bytes

    def _merge_ldw_halves(insts):
        """Fold row-tiled Ldweights pairs ([64,128] at row 0 + [64,128] at
        row 64 of the same tensor) into one [128,128] load carrying both
        halves' waits."""
        out = []
        pend = None  # (index_in_out, inst) of a candidate row-0 half
        for inst in insts:
            op = inst["opcode"]
            if inst.get("engine") != "PE":
                out.append(inst)
                continue
            if op == "Ldweights" and inst.get("tile_size") == [64, 128]:
                ap = inst["ins"][0].get("ap")
                if inst.get("tile_position") == [0, 0] and ap and ap[0][1] == 64:
                    out.append(inst)
                    pend = (len(out) - 1, inst)
                    continue
                if (pend is not None
                        and inst.get("tile_position") == [64, 0] and ap
                        and ap[0][1] == 64):
                    a = pend[1]
                    aap = a["ins"][0]["ap"]
                    same = (a["ins"][0].get("memref") == inst["ins"][0].get("memref")
                            and aap[0][0] == ap[0][0] and aap[1] == ap[1]
                            and inst["ins"][0].get("offset", 0)
                            == a["ins"][0].get("offset", 0) + 64 * aap[0][0])
                    b_si = inst.get("sync_info") or {}
                    if same and not b_si.get("on_update"):
                        aap[0][1] = 128
                        a["tile_size"] = [128, 128]
                        a.setdefault("sync_info", {"on_update": [], "on_wait": []})
                        a["sync_info"].setdefault("on_wait", [])
                        a["sync_info"]["on_wait"].extend(b_si.get("on_wait") or [])
                        pend = None
                        continue
                out.append(inst)
                pend = None
            else:
                if op not in ("Matmult", "NoOp"):
                    pend = None
                out.append(inst)
        return out

    def patched(self, *a, **k):
        d = json.loads(orig(self, *a, **k))
        for f in d["functions"]:
            for bb in f["blocks"]:
                bb["instructions"] = _merge_ldw_halves(bb["instructions"])
                out = []
                last_ldw = None  # (key, still_valid)
                for inst in bb["instructions"]:
                    si = inst.get("sync_info")
                    ow = (si or {}).get("on_wait") or []
                    op = inst["opcode"]

                    def emit_carriers(waits):
                        for j, w in enumerate(waits):
                            out.append({
                                "name": f"{inst['name']}__w{j}",
                                "opcode": "NoOp",
                                "engine": inst["engine"],
                                "ins": [], "outs": [],
                                "debug": inst.get("debug", 0),
                                "sync_info": {"on_update": [], "on_wait": [w]},
                            })

                    # drop a Ldweights identical to the previous one when only
                    # Matmult/NoOp sit between (weights already resident);
                    # also fold the row-tiled [64,128]+[64,128] half-pair into
                    # the single [128,128] load emitted by _merge_ldw_halves
                    if op == "Ldweights" and inst["engine"] == "PE":
                        key = json.dumps(
                            [inst.get("ins"), inst.get("tile_position"),
                             inst.get("tile_size")], sort_keys=True)
                        if last_ldw == key and not (si or {}).get("on_update"):
                            emit_carriers(ow)
                            continue
                        last_ldw = key
                    elif inst["engine"] == "PE" and op not in ("Matmult", "NoOp"):
                        last_ldw = None

                    if len(ow) > 1:
                        emit_carriers(ow[:-1])
                        si["on_wait"] = [ow[-1]]
                    out.append(inst)
                bb["instructions"] = out
        return json.dumps(d).encode()

    bass.Bass.to_json_bytes = patched


def build_kernel():
    nc = bass.Bass()
    xT = nc.declare_dram_parameter("xT", [DIM, SEQ], BF16, isOutput=False)
    wq = nc.declare_dram_parameter("wq", [DIM, FPC], BF16, isOutput=False)
    wk = nc.declare_dram_parameter("wk", [DIM, FPC], BF16, isOutput=False)
    wv = nc.declare_dram_parameter("wv", [DIM, FPC], BF16, isOutput=False)
    wo = nc.declare_dram_parameter("wo", [FPC, DIM], BF16, isOutput=False)
    # causal keep mask for the 128x128 diagonal block: kl <= ql
    tri = nc.declare_dram_parameter("tri", [128, 128], BF16, isOutput=False)
    outT = nc.declare_dram_parameter("outT", [DIM, SEQ], BF16, isOutput=True)

    with tile.TileContext(nc) as tc, ExitStack() as ctx:
        persist = ctx.enter_context(tc.tile_pool(name="persist", bufs=1))
        work = ctx.enter_context(tc.tile_pool(name="work", bufs=2))
        pt_pool = ctx.enter_context(tc.tile_pool(name="pt", bufs=1))
        ps_mm = ctx.enter_context(tc.tile_pool(name="ps_mm", bufs=2, space="PSUM"))
        ps_s = ctx.enter_context(tc.tile_pool(name="ps_s", bufs=2, space="PSUM"))
        ps_o = ctx.enter_context(tc.tile_pool(name="ps_o", bufs=2, space="PSUM"))

        # ---- load inputs. wq+xT feed the first matmuls: interleave them on
        # gpsimd so MM(di) can start after ~2 tiles; spread the rest across
        # otherwise-idle engine queues.
        w_sb = {"wq": [], "wk": [], "wv": []}
        xT_sb = []
        qs = [nc.gpsimd, nc.scalar, nc.sync]   # 3 parallel DMA queues
        # priority order: the chunk-0 critical set (wq + xT first halves)
        # lands first, then wk (needed by the K projections from ~10us),
        # then xT second halves, then wv/wo/tri.
        for di in range(8):
            eng = qs[di % 3]
            t = persist.tile([128, FPC], BF16, tag=f"wq{di}", name=f"wq{di}")
            eng.dma_start(t[:], wq.ap()[di * 128:(di + 1) * 128, :])
            w_sb["wq"].append(t)
            t = persist.tile([128, SEQ], BF16, tag=f"xT{di}", name=f"xT{di}")
            eng.dma_start(t[:, 0:1024],
                          xT.ap()[di * 128:(di + 1) * 128, 0:1024])
            xT_sb.append(t)
        for di in range(8):
            t = persist.tile([128, FPC], BF16, tag=f"wk{di}", name=f"wk{di}")
            qs[di % 3].dma_start(t[:], wk.ap()[di * 128:(di + 1) * 128, :])
            w_sb["wk"].append(t)
        for di in range(8):
            qs[di % 3].dma_start(xT_sb[di][:, 1024:2048],
                                 xT.ap()[di * 128:(di + 1) * 128, 1024:2048])
        for di in range(8):
            t = persist.tile([128, FPC], BF16, tag=f"wv{di}", name=f"wv{di}")
            nc.sync.dma_start(t[:], wv.ap()[di * 128:(di + 1) * 128, :])
            w_sb["wv"].append(t)
        wo_sb = []
        for fi in range(4):
            t = persist.tile([128, DIM], BF16, tag=f"wo{fi}")
            nc.gpsimd.dma_start(t[:], wo.ap()[fi * 128:(fi + 1) * 128, :])
            wo_sb.append(t)
        tri_sb = persist.tile([128, 128], BF16, tag="tri")
        nc.gpsimd.dma_start(tri_sb[:], tri.ap())

        # constants + the per-head ones columns in V (set once, upfront)
        ones64 = persist.tile([1, DH], BF16, tag="ones64")
        nc.gpsimd.memset(ones64[:], 1.0)
        v_sb = [persist.tile([128, HPC * VSTRIDE], BF16, tag=f"v{ti}",
                             name=f"v{ti}") for ti in range(NKT)]
        for ti in range(NKT):
            nc.gpsimd.memset(
                v_sb[ti][:].rearrange("p (h c) -> p h c", h=HPC)[:, :, DH:DH + 1],
                1.0)

        qk_sb = {"q": [], "k": []}
        for qn in ("q", "k"):
            for fi in range(4):
                qk_sb[qn].append(
                    persist.tile([128, SEQ], BF16, tag=f"{qn}{fi}",
                                 name=f"{qn}{fi}"))
        ot_sb = [persist.tile([128, SEQ], BF16, tag=f"ot{fi}", name=f"ot{fi}")
                 for fi in range(4)]
        pts_map = {}
        po_map = {}
        rc_map = {}

        def emit_qk(qn, wn, pr, parity=0):
            # Q, K in [feature, token] layout (w stationary, xT moving).
            # 4 simultaneous accumulators (2 ps_mm banks + the 2 halves of a
            # ps_s strip) keep weights stationary across 4 matmuls so the
            # LDW dedupe can drop 3 of 4. Alternating which tags lead
            # (parity) lets consecutive calls start before the previous
            # call's PSUM-drain copies finish.
            t = qk_sb[qn][pr]
            s_t = ps_s.tile([128, 1024], F32, tag="s", name="qkchain")
            mm_ch = [ps_mm.tile([128, 512], F32, tag="mm", name="ch0")[:],
                     ps_mm.tile([128, 512], F32, tag="mm", name="ch1")[:]]
            s_ch = [s_t[:][:, 0:512], s_t[:][:, 512:1024]]
            ch = mm_ch + s_ch if parity == 0 else s_ch + mm_ch
            for di in range(8):
                for tck in range(4):
                    nc.tensor.matmul(
                        ch[tck], w_sb[wn][di][:, pr * 128:(pr + 1) * 128],
                        xT_sb[di][:, tck * 512:(tck + 1) * 512],
                        start=(di == 0), stop=(di == 7))
            for tck in range(4):
                nc.vector.tensor_copy(t[:, tck * 512:(tck + 1) * 512], ch[tck])

        def emit_q_chunk(pr, tck):
            # Q projection for one 512-token chunk: an 8-matmul dense chain,
            # slotted into ACT-bound S phases (chunk tck's Q isn't needed
            # until chunk tck's attention)
            t = qk_sb["q"][pr]
            ch = ps_mm.tile([128, 512], F32, tag="mm", name="qch")
            for di in range(8):
                nc.tensor.matmul(
                    ch[:], w_sb["wq"][di][:, pr * 128:(pr + 1) * 128],
                    xT_sb[di][:, tck * 512:(tck + 1) * 512],
                    start=(di == 0), stop=(di == 7))
            nc.vector.tensor_copy(t[:, tck * 512:(tck + 1) * 512], ch[:])

        def emit_v(ti):
            # V in [token, feature] layout (xT stationary, wv moving), strided
            # into VSTRIDE-blocks; the ones columns were preset at startup
            p = ps_mm.tile([128, 512], F32, tag="mm", name="p_v")
            for di in range(8):
                nc.tensor.matmul(
                    p[:], xT_sb[di][:, ti * 128:(ti + 1) * 128],
                    w_sb["wv"][di][:],
                    start=(di == 0), stop=(di == 7))
            dst = v_sb[ti][:].rearrange("p (h c) -> p h c", h=HPC)[:, :, 0:DH]
            src = p[:].rearrange("p (h c) -> p h c", h=HPC)
            nc.vector.tensor_copy(dst, src)

        def emit_att_s(pr, ci):
            # S^T strips + exp into pt tiles for (head pair pr, q-chunk ci).
            # Diagonal tiles (r >= 0) only stream/exp their valid columns;
            # the 128-wide diagonal block is masked on gpsimd.
            q0 = ci * 512
            pts = pts_map[(pr, ci)] = [None] * (4 * ci + 4)
            # diagonal tiles first: their exps + gpsimd mask muls then finish
            # during the long non-diagonal stretch, so the PV chains (which
            # consume the diagonal tiles last) never wait on the masks.
            # Mid-phase, the next chunk's Q-projection block gives the PE
            # dense work while ACT's exp stream catches up.
            order = list(range(4 * ci, 4 * ci + 4)) + list(range(4 * ci))
            for idx, j in enumerate(order):
                if idx == min(6, len(order) - 1) and ci < 3:
                    emit_q_chunk(pr, ci + 1)
                r = j - 4 * ci
                c0 = 128 * r if r > 0 else 0
                ps = ps_s.tile([128, 1024], F32, tag="s", name="ps_st")
                for half in range(2):   # head A / head B, row-tiled
                    nc.tensor.matmul(
                        ps[:, half * 512 + c0:(half + 1) * 512],
                        qk_sb["k"][pr][half * 64:(half + 1) * 64,
                                       j * 128:(j + 1) * 128],
                        qk_sb["q"][pr][half * 64:(half + 1) * 64,
                                       q0 + c0:q0 + 512],
                        start=True, stop=True)
                pt = pt_pool.tile([128, 1024], BF16, tag=f"pt{j}", name="pt",
                                  bufs=2 if j < 12 else 1)
                pts[j] = pt
                if r < 0:
                    nc.scalar.activation(pt[:], ps[:], Exp, scale=SCALE)
                else:
                    pt3 = pt[:].rearrange("p (b w) -> p b w", b=2)[:, :, c0:]
                    ps3 = ps[:].rearrange("p (b w) -> p b w", b=2)[:, :, c0:]
                    nc.scalar.activation(pt3, ps3, Exp, scale=SCALE)
                    blk = pt[:].rearrange("p (b w) -> p b w", b=2)[:, :, c0:c0 + 128]
                    m3 = tri_sb[:][:, None, :].broadcast_to([128, 2, 128])
                    nc.gpsimd.tensor_mul(blk, blk, m3)

        def emit_att_pv(pr, ci):
            # PV: V_aug stationary [128k, 65], P^T moving, trimmed on the
            # diagonal. Output O^T_aug [65, 512q]: rows 0:64 = O^T, row 64 =
            # softmax sums. The half-0 reciprocal (ACT Ln->Exp, same table
            # set as the exps so no ACT_TABLE_LOAD) hides under the half-1
            # matmul chain; normalization is deferred to emit_finish.
            pts = pts_map.pop((pr, ci))
            po_map[(pr, ci)] = pos = []
            rc = work.tile([1, 1024], BF16, tag="rc", name="rc")
            for half in range(2):
                h = 2 * pr + half
                po = ps_o.tile([DH + 1, 512], F32, tag="o", name="po")
                pos.append(po)
                for j in range(4 * ci + 4):
                    r = j - 4 * ci
                    c0 = 128 * r if r > 0 else 0
                    nc.tensor.matmul(
                        po[:, c0:512],
                        v_sb[j][:, h * VSTRIDE:h * VSTRIDE + DH + 1],
                        pts[j][:, half * 512 + c0:(half + 1) * 512],
                        start=(j == 0), stop=(j == 4 * ci + 3))
                # 1/sums as exp(-ln(sums))
                l32 = work.tile([1, 512], F32, tag="l32", name="l32")
                nc.scalar.activation(l32[:], po[DH:DH + 1, :],
                                     mybir.ActivationFunctionType.Ln)
                nc.scalar.activation(rc[0:1, half * 512:(half + 1) * 512],
                                     l32[:], Exp, scale=-1.0)
            rc_map[(pr, ci)] = rc

        def emit_finish(pr, ci):
            # softmax normalization for pair pr: two rank-1 matmuls broadcast
            # the recip rows to [64,512], then two DVE muls write the
            # normalized O^T strips. Emitted one pair late so every input
            # is long since ready when the PE reaches the rb matmuls.
            q0 = ci * 512
            po_h0, po_h1 = po_map.pop((pr, ci))
            rc = rc_map.pop((pr, ci))
            for half, po in ((0, po_h0), (1, po_h1)):
                rb_ps = ps_mm.tile([DH, 512], F32, tag="mm", name="rb_ps")
                nc.tensor.matmul(rb_ps[:], ones64[:],
                                 rc[0:1, half * 512:(half + 1) * 512],
                                 start=True, stop=True)
                rb = work.tile([DH, 512], BF16, tag="rb", name="rb")
                nc.vector.tensor_copy(rb[:], rb_ps[:])
                nc.vector.tensor_mul(
                    ot_sb[pr][half * 64:(half + 1) * 64, q0:q0 + 512],
                    po[0:DH, :], rb[:])

        def emit_proj(ci):
            # projection for chunk ci's columns (all pairs' OT rows ready);
            # bf16 partials stored, host sums the two cores' halves in f32
            for ei in range(8):
                p = ps_mm.tile([128, 512], F32, tag="mm", name="p_proj")
                for fi in range(4):
                    nc.tensor.matmul(
                        p[:], wo_sb[fi][:, ei * 128:(ei + 1) * 128],
                        ot_sb[fi][:, ci * 512:(ci + 1) * 512],
                        start=(fi == 0), stop=(fi == 3))
                os_ = work.tile([128, 512], BF16, tag="os", name="os")
                nc.vector.tensor_copy(os_[:], p[:])
                nc.sync.dma_start(
                    outT.ap()[ei * 128:(ei + 1) * 128,
                              ci * 512:(ci + 1) * 512], os_[:])

        # Woven schedule. emit_finish(pr-1) leads pair pr: its rb matmuls
        # run right after PV(pr-1) (recips already done on ACT), and the DVE
        # norm muls complete during S(pr) so PV(pr)'s po buffers are free
        # the moment its chains start.
        for ci in range(NQC):
            for pr in range(4):
                if ci == 0:
                    emit_q_chunk(pr, 0)
                    if pr > 0:
                        emit_finish(pr - 1, ci)
                    emit_qk("k", "wk", pr, parity=1)
                elif pr > 0:
                    emit_finish(pr - 1, ci)
                emit_att_s(pr, ci)
                if pr == 0:
                    for ti in range(4 * ci, 4 * ci + 4):
                        emit_v(ti)
                emit_att_pv(pr, ci)
            emit_finish(3, ci)
            emit_proj(ci)
    return nc


_NC = None


def _get_nc():
    global _NC
    if _NC is None:
        _patch_walrus_wait_limit()
        _NC = build_kernel()
    return _NC


def _host_tri():
    kl = np.arange(128)[:, None]
    ql = np.arange(128)[None, :]
    return (kl <= ql).astype(ml_dtypes.bfloat16)


def kernel(x, w_qkv, w_out, _trace=False, _trace_kwargs=None):
    x = np.asarray(x, dtype=np.float32)
    w_qkv = np.asarray(w_qkv, dtype=np.float32)
    w_out = np.asarray(w_out, dtype=np.float32)
    nc = _get_nc()

    tri = _host_tri()
    in_maps = []
    for c in range(NCORES):
        b, g = c // 2, c % 2
        cols = slice(g * FPC, (g + 1) * FPC)
        in_maps.append({
            "xT": np.ascontiguousarray(x[b].T).astype(ml_dtypes.bfloat16),
            "wq": w_qkv[:, 0 * DIM:1 * DIM][:, cols].astype(ml_dtypes.bfloat16),
            "wk": w_qkv[:, 1 * DIM:2 * DIM][:, cols].astype(ml_dtypes.bfloat16),
            "wv": w_qkv[:, 2 * DIM:3 * DIM][:, cols].astype(ml_dtypes.bfloat16),
            "wo": w_out[g * FPC:(g + 1) * FPC, :].astype(ml_dtypes.bfloat16),
            "tri": tri,
        })

    res = run_bass_kernel_spmd(
        nc, in_maps, core_ids=list(range(NCORES)),
        trace=_trace, **(_trace_kwargs or {}))
    out = np.empty((4, SEQ, DIM), dtype=np.float32)
    for b in range(4):
        out[b] = (res.results[2 * b]["outT"].astype(np.float32)
                  + res.results[2 * b + 1]["outT"].astype(np.float32)).T
    if _trace:
        kernel.last_results = res
    return out
